# revision 3
# baseline (speedup 1.0000x reference)
"""DeltaSynapse message-passing kernel for Trainium2 (8 NeuronCores).

Computes I = einsum('eo,dbe,deo,dbe->bo', signs*W, Xd, delaymap, Wshort+1)
with the post dimension (o) sharded across 8 cores.

Math: reference signs = where(W>0, 2*signs_pre-1, 0) and W >= 0, so
signs*W == s*W with s = 2*signs_pre-1 (where W==0 both sides are 0). The
sign s[e] is folded into the host-side layout of Xd (Xd_signed = Xd*s),
so on device:
    I[b,o] = sum_{d,e} [(Wshort+1)*Xd_signed][d,b,e] * (delaymap*W)[d,e,o]

Per-core plan (o-shard of 256 columns), d-major streaming:
  - delaymap shard is binary -> stored fp8e4 in HBM (exact) and streamed
    by gpsimd SWDGE cast-DMAs (fp8 -> bf16) in 4 pair-of-delay chunks
    (1 MB HBM each). All cast-DMA descriptor generation is issued ahead
    of any Pool compute so the ring never stalls on a semaphore.
  - Queue placement (measured): the SWDGE ring sustains ~570 GB/s
    SBUF-side and carries w + the delaymap stream; the slow HWDGE
    queues carry only aux (scalar) and the 16 KB y writeback (sync).
    The y writeback's matmul-wait must not gate any other transfer, so
    sync carries nothing else.
  - m[d] = delaymap[d] * W as flat [P, 4096] bf16 tensor_tensors on the
    DVE: flat single-dim APs engage the DVE 2x packed mode (~224 G
    elem/s measured; sliced 3-dim views run at 1x).
  - A' = (Wshort+1)*Xd_signed is one fused DVE scalar_tensor_tensor.
  - PE: 128 bf16 matmuls (K=128 e's, M=16 batch, N=256 posts) on two
    interleaved PSUM accumulation chains (even/odd e-chunk), hiding the
    ~70 ns per-matmul latency bubble (154 vs 208 ns/matmul measured);
    DVE combines the two chains and sync DMAs the result out.
"""

import numpy as np

import concourse.bass as bass  # noqa: F401
import concourse.mybir as mybir
from concourse import bacc
from concourse.bass_utils import run_bass_kernel_spmd
from concourse.tile import TileContext

D, B, E, O = 8, 16, 2048, 2048
NCORES = 8
P = 128
O_SH = O // NCORES  # 256 post columns per core
EC = E // P  # 16 e-chunks
PAIRS = D // 2  # delaymap DMA granularity: 2 delays per transfer

_NC_CACHE = {}


def _build(loop_iters=None):
    f32 = mybir.dt.float32
    bf16 = mybir.dt.bfloat16
    fp8 = mybir.dt.float8e4

    nc = bacc.Bacc("TRN2", target_bir_lowering=False, debug=False)
    x_dm = nc.dram_tensor(
        "dm", [PAIRS, P, 2 * EC * O_SH], fp8, kind="ExternalInput"
    )
    x_w = nc.dram_tensor("w", [P, EC * O_SH], bf16, kind="ExternalInput")
    x_aux = nc.dram_tensor(
        "aux", [P, 2, EC, D * B], bf16, kind="ExternalInput"
    )
    y = nc.dram_tensor("y", [B, O_SH], f32, kind="ExternalOutput")

    with TileContext(nc) as tc:
        with (
            tc.tile_pool(name="const", bufs=3) as const,
            tc.tile_pool(name="dmp", bufs=3) as dmp,
            tc.tile_pool(name="mp", bufs=8) as mp,
            tc.tile_pool(name="psp", bufs=2, space="PSUM") as psp,
            tc.tile_pool(name="outp", bufs=2) as outp,
        ):

            def body(_i=None):
                aux_t = const.tile([P, 2, EC, D * B], bf16)
                w_t = const.tile([P, EC, O_SH], bf16)
                # ring: w first (every multiply consumes it), then the
                # delaymap stream; scalar: aux; sync: y only
                nc.gpsimd.dma_start(out=w_t[:], in_=x_w.ap())
                nc.scalar.dma_start(out=aux_t[:], in_=x_aux.ap())

                dm_ts = []
                for g in range(PAIRS):
                    dm_t = dmp.tile([P, 2, EC, O_SH], bf16, tag="dm")
                    nc.gpsimd.dma_start(out=dm_t[:], in_=x_dm.ap()[g])
                    dm_ts.append(dm_t)

                # A' = (Wshort + 1) * (Xd * s)  (sign pre-folded on host)
                a_r = const.tile([P, EC, D, B], bf16)
                nc.vector.scalar_tensor_tensor(
                    a_r[:].rearrange("p c d b -> p (c d b)"),
                    aux_t[:, 1].rearrange("p c x -> p (c x)"),
                    1.0,
                    aux_t[:, 0].rearrange("p c x -> p (c x)"),
                    mybir.AluOpType.add,
                    mybir.AluOpType.mult,
                )

                ps0 = psp.tile([B, O_SH], f32, name="ps0", tag="ps0")
                ps1 = psp.tile([B, O_SH], f32, name="ps1", tag="ps1")
                for g in range(PAIRS):
                    dm_t = dm_ts[g]
                    for j in range(2):
                        d = 2 * g + j
                        m_t = mp.tile([P, EC, O_SH], bf16, tag="m")
                        # flat APs: DVE 2x packed mode needs single-dim
                        # free access patterns
                        nc.vector.tensor_tensor(
                            m_t[:].rearrange("p c o -> p (c o)"),
                            dm_t[:, j].rearrange("p c o -> p (c o)"),
                            w_t[:].rearrange("p c o -> p (c o)"),
                            mybir.AluOpType.mult,
                        )
                        for c in range(EC):
                            nc.tensor.matmul(
                                (ps0 if c % 2 == 0 else ps1)[:],
                                a_r[:, c, d, :],
                                m_t[:, c, :],
                                start=(d == 0 and c < 2),
                                stop=(d == D - 1 and c >= EC - 2),
                            )

                o_t = outp.tile([B, O_SH], f32)
                # chain-combine on DVE (idle at iteration end; an ACT copy
                # here would gate the next aux issue behind the matmuls)
                nc.vector.tensor_copy(out=o_t[:], in_=ps0[:])
                nc.vector.tensor_tensor(
                    o_t[:], o_t[:], ps1[:], mybir.AluOpType.add
                )
                nc.sync.dma_start(out=y.ap(), in_=o_t[:])

            if loop_iters is None:
                body()
            else:
                with tc.For_i(
                    0, loop_iters, 1, hint_engines=(mybir.EngineType.PE,)
                ) as i:
                    body(i)

    nc.compile()
    return nc


def _get_nc(loop_iters=None):
    if loop_iters not in _NC_CACHE:
        _NC_CACHE[loop_iters] = _build(loop_iters)
    return _NC_CACHE[loop_iters]


def _make_in_maps(W, Xd, delaymap, Wshort, signs_pre):
    import ml_dtypes

    bf16 = ml_dtypes.bfloat16
    fp8 = ml_dtypes.float8_e4m3fn
    W = np.asarray(W, dtype=np.float32)
    Xd = np.asarray(Xd, dtype=np.float32)
    Wshort = np.asarray(Wshort, dtype=np.float32)
    signs_pre = np.asarray(signs_pre)

    s = (2 * signs_pre - 1).astype(np.float32)  # (E,)
    s_re = s.reshape(EC, P).T  # (P, EC)
    # (P, EC, D, B) views of Xd/Wshort; sign folded into Xd
    xd_re = Xd.reshape(D, B, EC, P).transpose(3, 2, 0, 1)
    xd_re = xd_re * s_re[:, :, None, None]
    ws_re = Wshort.reshape(D, B, EC, P).transpose(3, 2, 0, 1)
    aux_re = np.ascontiguousarray(
        np.stack([xd_re, ws_re], axis=1).astype(bf16)
    ).reshape(P, 2, EC, D * B)

    # binary delaymap -> fp8e4m3 via byte trick (0.0 -> 0x00, 1.0 -> 0x38):
    # exact and ~10x faster than a float cast on the host
    dm8 = (np.asarray(delaymap) != 0).astype(np.uint8) * np.uint8(0x38)

    in_maps = []
    for i in range(NCORES):
        o0 = i * O_SH
        w_re = np.ascontiguousarray(
            W[:, o0 : o0 + O_SH]
            .reshape(EC, P, O_SH)
            .transpose(1, 0, 2)
            .reshape(P, EC * O_SH)
            .astype(bf16)
        )
        # (PAIRS, P, 2*EC*O_SH): per pair-of-delays, per-partition rows
        # hold both delays' (EC, O_SH) blocks contiguously
        dm_re = (
            np.ascontiguousarray(
                dm8[:, :, o0 : o0 + O_SH]
                .reshape(PAIRS, 2, EC, P, O_SH)
                .transpose(0, 3, 1, 2, 4)
                .reshape(PAIRS, P, 2 * EC * O_SH)
            )
            .view(fp8)
        )
        in_maps.append({"dm": dm_re, "w": w_re, "aux": aux_re})
    return in_maps


def run(W, Xd, delaymap, Wshort, signs_pre, loop_iters=None):
    """Run on the 8 NeuronCores; returns (I, BassKernelResults)."""
    nc = _get_nc(loop_iters)
    in_maps = _make_in_maps(W, Xd, delaymap, Wshort, signs_pre)
    res = run_bass_kernel_spmd(nc, in_maps, core_ids=list(range(NCORES)))
    I = np.concatenate(
        [res.results[i]["y"] for i in range(NCORES)], axis=1
    ).astype(np.float32)
    return I, res


def kernel(W, Xd, delaymap, Wshort, signs_pre):
    I, _ = run(W, Xd, delaymap, Wshort, signs_pre)
    return I


# revision 5
# speedup vs baseline: 1.1805x; 1.1805x over previous
"""DeltaSynapse message-passing kernel for Trainium2 (8 NeuronCores).

Computes I = einsum('eo,dbe,deo,dbe->bo', signs*W, Xd, delaymap, Wshort+1)
with the post dimension (o) sharded across 8 cores.

Math: reference signs = where(W>0, 2*signs_pre-1, 0) and W >= 0, so
signs*W == s*W with s = 2*signs_pre-1 (where W==0 both sides are 0). The
sign s[e] is folded into the host-side layout of Xd (Xd_signed = Xd*s),
so on device:
    I[b,o] = sum_{d,e} [(Wshort+1)*Xd_signed][d,b,e] * (delaymap*W)[d,e,o]

Per-core plan (o-shard of 256 columns), d-major streaming:
  - delaymap shard is binary -> stored fp8e4 in HBM (exact) and streamed
    by gpsimd SWDGE cast-DMAs (fp8 -> bf16) in 4 pair-of-delay chunks
    (1 MB HBM each). All cast-DMA descriptor generation is issued ahead
    of any Pool compute so the ring never stalls on a semaphore.
  - Queue placement (measured): the SWDGE ring sustains ~570 GB/s
    SBUF-side and carries w + the delaymap stream; the slow HWDGE
    queues carry only aux (scalar) and the 16 KB y writeback (sync).
    The y writeback's matmul-wait must not gate any other transfer, so
    sync carries nothing else.
  - m[d] = delaymap[d] * W as flat [P, 4096] bf16 tensor_tensors on the
    DVE: flat single-dim APs engage the DVE 2x packed mode (~224 G
    elem/s measured; sliced 3-dim views run at 1x).
  - A' = (Wshort+1)*Xd_signed is one fused DVE scalar_tensor_tensor.
  - PE: 128 bf16 matmuls (K=128 e's, M=16 batch, N=256 posts) on two
    interleaved PSUM accumulation chains (even/odd e-chunk), hiding the
    ~70 ns per-matmul latency bubble (154 vs 208 ns/matmul measured);
    DVE combines the two chains and sync DMAs the result out.
"""

import numpy as np

import concourse.bass as bass  # noqa: F401
import concourse.mybir as mybir
from concourse import bacc
from concourse.bass_utils import run_bass_kernel_spmd
from concourse.tile import TileContext

D, B, E, O = 8, 16, 2048, 2048
NCORES = 8
P = 128
O_SH = O // NCORES  # 256 post columns per core
EC = E // P  # 16 e-chunks
PAIRS = D // 2  # delaymap DMA granularity: 2 delays per transfer

_NC_CACHE = {}


def _build(loop_iters=None):
    f32 = mybir.dt.float32
    bf16 = mybir.dt.bfloat16
    fp8 = mybir.dt.float8e4

    nc = bacc.Bacc("TRN2", target_bir_lowering=False, debug=False)
    x_dm = nc.dram_tensor(
        "dm", [PAIRS, P, 2 * EC * O_SH], fp8, kind="ExternalInput"
    )
    x_w = nc.dram_tensor("w", [P, EC * O_SH], bf16, kind="ExternalInput")
    x_aux = nc.dram_tensor(
        "aux", [P, 2, EC, D * B], bf16, kind="ExternalInput"
    )
    y = nc.dram_tensor("y", [B, O_SH], f32, kind="ExternalOutput")

    with TileContext(nc) as tc:
        with (
            tc.tile_pool(name="const", bufs=3) as const,
            tc.tile_pool(name="dmp", bufs=5) as dmp,
            tc.tile_pool(name="mp", bufs=6) as mp,
            tc.tile_pool(name="psp", bufs=2, space="PSUM") as psp,
            tc.tile_pool(name="outp", bufs=2) as outp,
        ):

            def body(_i=None):
                aux_t = const.tile([P, 2, EC, D * B], bf16)
                w_t = const.tile([P, EC, O_SH], bf16)
                # ring: w first (every multiply consumes it), then the
                # delaymap stream; scalar: aux; sync: y only
                nc.gpsimd.dma_start(out=w_t[:], in_=x_w.ap())
                nc.scalar.dma_start(out=aux_t[:], in_=x_aux.ap())

                dm_ts = []
                for g in range(PAIRS):
                    dm_t = dmp.tile([P, 2, EC, O_SH], bf16, tag="dm")
                    nc.gpsimd.dma_start(out=dm_t[:], in_=x_dm.ap()[g])
                    dm_ts.append(dm_t)

                # A' = (Wshort + 1) * (Xd * s)  (sign pre-folded on host)
                a_r = const.tile([P, EC, D, B], bf16)
                nc.vector.scalar_tensor_tensor(
                    a_r[:].rearrange("p c d b -> p (c d b)"),
                    aux_t[:, 1].rearrange("p c x -> p (c x)"),
                    1.0,
                    aux_t[:, 0].rearrange("p c x -> p (c x)"),
                    mybir.AluOpType.add,
                    mybir.AluOpType.mult,
                )

                ps0 = psp.tile([B, O_SH], f32, name="ps0", tag="ps0")
                ps1 = psp.tile([B, O_SH], f32, name="ps1", tag="ps1")
                for g in range(PAIRS):
                    dm_t = dm_ts[g]
                    for j in range(2):
                        d = 2 * g + j
                        m_t = mp.tile([P, EC, O_SH], bf16, tag="m")
                        # flat APs: DVE 2x packed mode needs single-dim
                        # free access patterns
                        nc.vector.tensor_tensor(
                            m_t[:].rearrange("p c o -> p (c o)"),
                            dm_t[:, j].rearrange("p c o -> p (c o)"),
                            w_t[:].rearrange("p c o -> p (c o)"),
                            mybir.AluOpType.mult,
                        )
                        for c in range(EC):
                            nc.tensor.matmul(
                                (ps0 if c % 2 == 0 else ps1)[:],
                                a_r[:, c, d, :],
                                m_t[:, c, :],
                                start=(d == 0 and c < 2),
                                stop=(d == D - 1 and c >= EC - 2),
                            )

                o_t = outp.tile([B, O_SH], f32)
                # chain-combine on DVE (idle at iteration end; an ACT copy
                # here would gate the next aux issue behind the matmuls)
                nc.vector.tensor_copy(out=o_t[:], in_=ps0[:])
                nc.vector.tensor_tensor(
                    o_t[:], o_t[:], ps1[:], mybir.AluOpType.add
                )
                nc.sync.dma_start(out=y.ap(), in_=o_t[:])

            if loop_iters is None:
                body()
            else:
                with tc.For_i(
                    0, loop_iters, 1, hint_engines=(mybir.EngineType.PE,)
                ) as i:
                    body(i)

    nc.compile()
    return nc


def _get_nc(loop_iters=None):
    if loop_iters not in _NC_CACHE:
        _NC_CACHE[loop_iters] = _build(loop_iters)
    return _NC_CACHE[loop_iters]


def _make_in_maps(W, Xd, delaymap, Wshort, signs_pre):
    import ml_dtypes

    bf16 = ml_dtypes.bfloat16
    fp8 = ml_dtypes.float8_e4m3fn
    W = np.asarray(W, dtype=np.float32)
    Xd = np.asarray(Xd, dtype=np.float32)
    Wshort = np.asarray(Wshort, dtype=np.float32)
    signs_pre = np.asarray(signs_pre)

    s = (2 * signs_pre - 1).astype(np.float32)  # (E,)
    s_re = s.reshape(EC, P).T  # (P, EC)
    # (P, EC, D, B) views of Xd/Wshort; sign folded into Xd
    xd_re = Xd.reshape(D, B, EC, P).transpose(3, 2, 0, 1)
    xd_re = xd_re * s_re[:, :, None, None]
    ws_re = Wshort.reshape(D, B, EC, P).transpose(3, 2, 0, 1)
    aux_re = np.ascontiguousarray(
        np.stack([xd_re, ws_re], axis=1).astype(bf16)
    ).reshape(P, 2, EC, D * B)

    # binary delaymap -> fp8e4m3 via byte trick (0.0 -> 0x00, 1.0 -> 0x38):
    # exact and ~10x faster than a float cast on the host
    dm8 = (np.asarray(delaymap) != 0).astype(np.uint8) * np.uint8(0x38)

    in_maps = []
    for i in range(NCORES):
        o0 = i * O_SH
        w_re = np.ascontiguousarray(
            W[:, o0 : o0 + O_SH]
            .reshape(EC, P, O_SH)
            .transpose(1, 0, 2)
            .reshape(P, EC * O_SH)
            .astype(bf16)
        )
        # (PAIRS, P, 2*EC*O_SH): per pair-of-delays, per-partition rows
        # hold both delays' (EC, O_SH) blocks contiguously
        dm_re = (
            np.ascontiguousarray(
                dm8[:, :, o0 : o0 + O_SH]
                .reshape(PAIRS, 2, EC, P, O_SH)
                .transpose(0, 3, 1, 2, 4)
                .reshape(PAIRS, P, 2 * EC * O_SH)
            )
            .view(fp8)
        )
        in_maps.append({"dm": dm_re, "w": w_re, "aux": aux_re})
    return in_maps


def run(W, Xd, delaymap, Wshort, signs_pre, loop_iters=None):
    """Run on the 8 NeuronCores; returns (I, BassKernelResults)."""
    nc = _get_nc(loop_iters)
    in_maps = _make_in_maps(W, Xd, delaymap, Wshort, signs_pre)
    res = run_bass_kernel_spmd(nc, in_maps, core_ids=list(range(NCORES)))
    I = np.concatenate(
        [res.results[i]["y"] for i in range(NCORES)], axis=1
    ).astype(np.float32)
    return I, res


def kernel(W, Xd, delaymap, Wshort, signs_pre):
    I, _ = run(W, Xd, delaymap, Wshort, signs_pre)
    return I
